# revision 4
# baseline (speedup 1.0000x reference)
"""Trainium2 Bass kernel: decoder multi-head attention (causal), batch-parallel
across 8 NeuronCores (one batch element per core).

Per-core layout strategy (S=1024, E=768, H=12, D=64):
  - inputs pre-transposed on host to [E, S]; per-head projections packed to
    [E+1, H*D] with the bias as an extra contraction row (K-augmentation).
  - QT/KT computed in [H*D, S] layout (head-major partitions, 2 heads/tile).
  - V computed in natural [S, H*(D+1)] layout with a ones column per head so
    the P@V' matmul also produces the softmax denominator (row 64 of psum).
  - scores computed transposed [t, s] per 128-row t-block; softmax skips the
    max subtraction (scores are small); exp on ScalarE with fused *SCALE;
    causal diagonal-block masking via GpSimd multiply with a triangular mask;
    fully-masked column ranges are simply never read by the P@V' matmuls.
  - denominator broadcast via a K=1 outer-product matmul, reciprocal via
    DVE reciprocal_approx_fast, per-head normalize of the [H*D, S] numerator.
  - output projection consumes the normalized [H*D, S] numerator directly
    (plus a K=1 bias row), writing Y = concat @ Wo + bo tile by tile.
  All matmuls run in float32r (1 col/cycle for N>=256 vs 4 for fp32).
"""

import numpy as np

import concourse.bass as bass
import concourse.mybir as mybir
import concourse.tile as tile
import concourse.bacc as bacc
from concourse import bass_utils

B, S, E, H, D = 8, 1024, 768, 12, 64
HD = H * D
P = 128
SCALE = 1.0 / np.float32(np.sqrt(D))
NKT = E // P     # 6 contraction chunks for the input projections
NMT = HD // P    # 6 head-major tiles (2 heads each)
NST = S // P     # 8 sequence tiles of 128
F32 = mybir.dt.float32
F32R = mybir.dt.float32r
EXP = mybir.ActivationFunctionType.Exp

_CACHE = {}


def _build():
    nc = bacc.Bacc("TRN2", debug=False, target_bir_lowering=False)

    xqT = nc.dram_tensor("xqT", [E, S], F32R, kind="ExternalInput").ap()
    xkT = nc.dram_tensor("xkT", [E, S], F32R, kind="ExternalInput").ap()
    xvT = nc.dram_tensor("xvT", [E, S], F32R, kind="ExternalInput").ap()
    wq = nc.dram_tensor("wq", [E + 1, HD], F32R, kind="ExternalInput").ap()
    wk = nc.dram_tensor("wk", [E + 1, HD], F32R, kind="ExternalInput").ap()
    wv = nc.dram_tensor("wv", [E + 1, HD], F32R, kind="ExternalInput").ap()
    wo = nc.dram_tensor("wo", [HD + 1, E], F32R, kind="ExternalInput").ap()
    tri = nc.dram_tensor("tri", [P, P], F32R, kind="ExternalInput").ap()
    ones_s = nc.dram_tensor("ones_s", [1, S], F32R, kind="ExternalInput").ap()
    ones_d = nc.dram_tensor("ones_d", [1, 64], F32R, kind="ExternalInput").ap()
    ones_c = nc.dram_tensor("ones_c", [P, H], F32R, kind="ExternalInput").ap()
    y = nc.dram_tensor("y", [S, E], F32, kind="ExternalOutput").ap()

    with tile.TileContext(nc) as tc:
        with (
            tc.tile_pool(name="cst", bufs=1) as cst,
            tc.tile_pool(name="xin", bufs=12) as xin,
            tc.tile_pool(name="win", bufs=12) as win,
            tc.tile_pool(name="wbias", bufs=3) as wbias,
            tc.tile_pool(name="qt", bufs=2) as qtp,
            tc.tile_pool(name="kt", bufs=2) as ktp,
            tc.tile_pool(name="vp", bufs=8) as vpp,
            tc.tile_pool(name="numt", bufs=6) as numtp,
            tc.tile_pool(name="expt", bufs=4) as exptp,
            tc.tile_pool(name="dst", bufs=2) as dstp,
            tc.tile_pool(name="dinv", bufs=2) as dinvp,
            tc.tile_pool(name="outsb", bufs=2) as outp,
            tc.tile_pool(name="projps", bufs=2, space="PSUM") as projps,
            tc.tile_pool(name="scps", bufs=2, space="PSUM") as scps,
            tc.tile_pool(name="pvps", bufs=2, space="PSUM") as pvps,
        ):
            # ---- constants ----
            onr = cst.tile([1, S], F32R, tag="onr")
            nc.sync.dma_start(onr[:], ones_s[:])
            on64 = cst.tile([P, 64], F32R, tag="on64")
            nc.sync.dma_start(on64[64:65, :], ones_d[:])
            trit = cst.tile([P, P], F32R, tag="trit")
            nc.sync.dma_start(trit[:], tri[:])

            # ---- V weights + inputs ----
            wv_sb = []
            for kk in range(NKT):
                t = win.tile([P, HD], F32R, tag="w")
                nc.sync.dma_start(t[:], wv[kk * P:(kk + 1) * P, :])
                wv_sb.append(t)
            wv_b = wbias.tile([1, HD], F32R, tag="wb")
            nc.sync.dma_start(wv_b[:], wv[E:E + 1, :])
            xv_sb = []
            for kk in range(NKT):
                t = xin.tile([P, S], F32R, tag="x")
                nc.sync.dma_start(t[:], xvT[kk * P:(kk + 1) * P, :])
                xv_sb.append(t)

            # ---- V projection -> vp tiles [S-tile, H*(D+1)] with ones cols ----
            vp_sb = []
            for mS in range(NST):
                vt = vpp.tile([P, H * (D + 1)], F32R, tag="v")
                vp_sb.append(vt)
                v3 = vt[:].rearrange("p (h e) -> p h e", e=D + 1)
                nc.sync.dma_start(v3[:, :, D:D + 1], ones_c[:])
                for n in range(2):
                    w = 512 if n == 0 else HD - 512
                    h0 = n * (512 // D)
                    ps = projps.tile([P, 512], F32, tag="pp")
                    for kk in range(NKT):
                        nc.tensor.matmul(
                            ps[:, :w],
                            xv_sb[kk][:, mS * P:(mS + 1) * P],
                            wv_sb[kk][:, n * 512:n * 512 + w],
                            start=(kk == 0), stop=False)
                    nc.tensor.matmul(
                        ps[:, :w],
                        onr[0:1, mS * P:(mS + 1) * P],
                        wv_b[0:1, n * 512:n * 512 + w],
                        start=False, stop=True)
                    nc.vector.tensor_copy(
                        v3[:, h0:h0 + w // D, 0:D], ps[:, :w])

            # ---- Q/K weights + inputs (loaded up front; slots recycle) ----
            wq_sb, wk_sb = [], []
            for kk in range(NKT):
                t = win.tile([P, HD], F32R, tag="w")
                nc.sync.dma_start(t[:], wq[kk * P:(kk + 1) * P, :])
                wq_sb.append(t)
            wq_b = wbias.tile([1, HD], F32R, tag="wb")
            nc.sync.dma_start(wq_b[:], wq[E:E + 1, :])
            xq_sb = []
            for kk in range(NKT):
                t = xin.tile([P, S], F32R, tag="x")
                nc.sync.dma_start(t[:], xqT[kk * P:(kk + 1) * P, :])
                xq_sb.append(t)
            for kk in range(NKT):
                t = win.tile([P, HD], F32R, tag="w")
                nc.sync.dma_start(t[:], wk[kk * P:(kk + 1) * P, :])
                wk_sb.append(t)
            wk_b = wbias.tile([1, HD], F32R, tag="wb")
            nc.sync.dma_start(wk_b[:], wk[E:E + 1, :])
            xk_sb = []
            for kk in range(NKT):
                t = xin.tile([P, S], F32R, tag="x")
                nc.sync.dma_start(t[:], xkT[kk * P:(kk + 1) * P, :])
                xk_sb.append(t)

            def qk_proj(m, w_sb, w_b, x_sb, out_t, n):
                """One [128, 512] psum tile of the QT/KT projection."""
                ps = projps.tile([P, 512], F32, tag="pp")
                for kk in range(NKT):
                    nc.tensor.matmul(
                        ps[:],
                        w_sb[kk][:, m * P:(m + 1) * P],
                        x_sb[kk][:, n * 512:(n + 1) * 512],
                        start=(kk == 0), stop=False)
                nc.tensor.matmul(
                    ps[:],
                    w_b[0:1, m * P:(m + 1) * P],
                    onr[0:1, n * 512:(n + 1) * 512],
                    start=False, stop=True)
                nc.vector.tensor_copy(out_t[:, n * 512:(n + 1) * 512], ps[:])

            numt_sb = [None] * NMT

            def attention_head(h, qt_t, kt_t, num_t, filler):
                """Causal attention for one head; filler() emits matmul work
                between stages to keep the PE busy while ACT runs exp."""
                off = (h % 2) * 64
                dinv = dinvp.tile([P, S], F32, tag="di")
                dtmp = dstp.tile([P, S], F32, tag="dt")
                for ci in range(2):
                    nblk = 4 * ci + 4
                    ets = []
                    # scores (transposed [t, s]) + exp, two t-blocks per tile
                    for tj in range(nblk // 2):
                        sp = scps.tile([P, 1024], F32, tag="sc")
                        et = exptp.tile([P, 1024], F32R, tag="et")
                        ets.append(et)
                        for b2 in range(2):
                            ti = tj * 2 + b2
                            voff = max(0, ti * P - ci * 512)
                            col0 = min(voff, 256)  # widen to >=256 for f32r
                            nc.tensor.matmul(
                                sp[:, b2 * 512 + col0:(b2 + 1) * 512],
                                kt_t[off:off + 64, ti * P:(ti + 1) * P],
                                qt_t[off:off + 64,
                                     ci * 512 + col0:(ci + 1) * 512],
                                start=True, stop=True)
                        nc.scalar.activation(et[:], sp[:], EXP, scale=float(SCALE))
                        for b2 in range(2):
                            ti = tj * 2 + b2
                            voff = ti * P - ci * 512
                            if 0 <= voff < 512:  # diagonal block: triangular mask
                                sl = et[:, b2 * 512 + voff:b2 * 512 + voff + P]
                                nc.gpsimd.tensor_mul(sl, sl, trit[:])
                    if filler:
                        filler()
                    # P @ [V | 1]  (accumulate over t-blocks; den in row 64)
                    pvp = pvps.tile([65, 512], F32, tag="pv")
                    for ti in range(nblk):
                        voff = max(0, ti * P - ci * 512)
                        et = ets[ti // 2]
                        nc.tensor.matmul(
                            pvp[:, voff:512],
                            vp_sb[ti][:, h * (D + 1):(h + 1) * (D + 1)],
                            et[:, (ti % 2) * 512 + voff:(ti % 2 + 1) * 512],
                            start=(ti == 0), stop=(ti == nblk - 1),
                            skip_group_check=(ti > 0))
                    # numerator -> numt rows [off:off+64]
                    nc.vector.tensor_copy(
                        num_t[off:off + 64, ci * 512:(ci + 1) * 512],
                        pvp[0:64, :])
                    # denominator: stage row, broadcast via K=1 outer product
                    dstg = dstp.tile([P, 512], F32R, tag="ds")
                    nc.vector.tensor_copy(dstg[64:65, :], pvp[64:65, :])
                    dps = projps.tile([P, 512], F32, tag="pp")
                    nc.tensor.matmul(
                        dps[0:64, :],
                        on64[64:65, 0:64],
                        dstg[64:65, :],
                        start=True, stop=True)
                    nc.vector.tensor_copy(
                        dtmp[0:64, ci * 512:(ci + 1) * 512], dps[0:64, :])
                nc.vector.reciprocal_approx_fast(
                    out=dinv[0:64, :], in_=dtmp[0:64, :])
                if off:
                    nc.vector.tensor_copy(dinv[64:128, :], dinv[0:64, :])
                # normalize this head's numerator rows
                nc.vector.tensor_mul(
                    num_t[off:off + 64, :], num_t[off:off + 64, :],
                    dinv[off:off + 64, :])

            # ---- interleaved Q/K projection + attention, pair by pair ----
            # pair 0 needs qt/kt tile 0 first
            qt_t = qtp.tile([P, S], F32R, tag="q")
            kt_t = ktp.tile([P, S], F32R, tag="k")
            for n in range(2):
                qk_proj(0, wq_sb, wq_b, xq_sb, qt_t, n)
            for n in range(2):
                qk_proj(0, wk_sb, wk_b, xk_sb, kt_t, n)
            for m in range(NMT):
                num_t = numtp.tile([P, S], F32R, tag="n")
                numt_sb[m] = num_t
                # filler work: next pair's projections, one psum tile at a time
                fill_jobs = []
                if m + 1 < NMT:
                    qt_n = qtp.tile([P, S], F32R, tag="q")
                    kt_n = ktp.tile([P, S], F32R, tag="k")
                    for n in range(2):
                        fill_jobs.append((qk_proj, m + 1, wq_sb, wq_b, xq_sb, qt_n, n))
                    for n in range(2):
                        fill_jobs.append((qk_proj, m + 1, wk_sb, wk_b, xk_sb, kt_n, n))

                def filler():
                    if fill_jobs:
                        f, *args = fill_jobs.pop(0)
                        f(*args)

                attention_head(2 * m, qt_t, kt_t, num_t, filler)
                attention_head(2 * m + 1, qt_t, kt_t, num_t, filler)
                while fill_jobs:
                    filler()
                if m + 1 < NMT:
                    qt_t, kt_t = qt_n, kt_n

            # ---- output projection ----
            wo_sb = []
            for kk in range(NMT):
                t = win.tile([P, E], F32R, tag="w")
                nc.sync.dma_start(t[:], wo[kk * P:(kk + 1) * P, :])
                wo_sb.append(t)
            wo_b = wbias.tile([1, E], F32R, tag="wb")
            nc.sync.dma_start(wo_b[:], wo[HD:HD + 1, :])
            for mS in range(NST):
                ot = outp.tile([P, E], F32, tag="o")
                for n in range(2):
                    w = 512 if n == 0 else E - 512
                    ps = projps.tile([P, 512], F32, tag="pp")
                    for kk in range(NMT):
                        nc.tensor.matmul(
                            ps[:, :w],
                            numt_sb[kk][:, mS * P:(mS + 1) * P],
                            wo_sb[kk][:, n * 512:n * 512 + w],
                            start=(kk == 0), stop=False)
                    nc.tensor.matmul(
                        ps[:, :w],
                        onr[0:1, mS * P:(mS + 1) * P],
                        wo_b[0:1, n * 512:n * 512 + w],
                        start=False, stop=True)
                    nc.vector.tensor_copy(ot[:, n * 512:n * 512 + w], ps[:, :w])
                nc.sync.dma_start(y[mS * P:(mS + 1) * P, :], ot[:])

    nc.compile()
    return nc


def _get_nc():
    if "nc" not in _CACHE:
        _CACHE["nc"] = _build()
    return _CACHE["nc"]


def kernel(queries, keys, values, attn_mask, Wq, bq, Wk, bk, Wv, bv, Wo, bo,
           **extra):
    nc = _get_nc()

    queries = np.asarray(queries, np.float32)
    keys = np.asarray(keys, np.float32)
    values = np.asarray(values, np.float32)
    wq_a = np.concatenate(
        [np.asarray(Wq, np.float32).transpose(1, 0, 2).reshape(E, HD),
         np.asarray(bq, np.float32).reshape(1, HD)], axis=0)
    wk_a = np.concatenate(
        [np.asarray(Wk, np.float32).transpose(1, 0, 2).reshape(E, HD),
         np.asarray(bk, np.float32).reshape(1, HD)], axis=0)
    wv_a = np.concatenate(
        [np.asarray(Wv, np.float32).transpose(1, 0, 2).reshape(E, HD),
         np.asarray(bv, np.float32).reshape(1, HD)], axis=0)
    wo_a = np.concatenate(
        [np.asarray(Wo, np.float32).reshape(HD, E),
         np.asarray(bo, np.float32).reshape(1, E)], axis=0)
    tri_m = np.triu(np.ones((P, P), np.float32))
    ones_s = np.ones((1, S), np.float32)
    ones_d = np.ones((1, 64), np.float32)
    ones_c = np.ones((P, H), np.float32)

    in_maps = []
    for b in range(B):
        in_maps.append({
            "xqT": np.ascontiguousarray(queries[b].T),
            "xkT": np.ascontiguousarray(keys[b].T),
            "xvT": np.ascontiguousarray(values[b].T),
            "wq": wq_a, "wk": wk_a, "wv": wv_a, "wo": wo_a,
            "tri": tri_m, "ones_s": ones_s, "ones_d": ones_d,
            "ones_c": ones_c,
        })
    res = bass_utils.run_bass_kernel_spmd(
        nc, in_maps, core_ids=list(range(B)), **extra)
    out = np.stack([res.results[c]["y"] for c in range(B)], axis=0)
    if extra:
        _CACHE["last_result"] = res
    return out


# revision 5
# speedup vs baseline: 1.3790x; 1.3790x over previous
"""Trainium2 Bass kernel: decoder multi-head attention (causal), batch-parallel
across 8 NeuronCores (one batch element per core).

Per-core layout strategy (S=1024, E=768, H=12, D=64):
  - inputs pre-transposed on host to [E, S]; per-head projections packed to
    [E+1, H*D] with the bias as an extra contraction row (K-augmentation).
  - QT/KT computed in [H*D, S] layout (head-major partitions, 2 heads/tile).
  - V computed in natural [S, H*(D+1)] layout with a ones column per head so
    the P@V' matmul also produces the softmax denominator (row 64 of psum).
  - scores computed transposed [t, s] per 128-row t-block; softmax skips the
    max subtraction (scores are small); exp on ScalarE with fused *SCALE;
    causal diagonal-block masking via GpSimd multiply with a triangular mask;
    fully-masked column ranges are simply never read by the P@V' matmuls.
  - denominator broadcast via a K=1 outer-product matmul, reciprocal via
    DVE reciprocal_approx_fast, per-head normalize of the [H*D, S] numerator.
  - output projection consumes the normalized [H*D, S] numerator directly
    (plus a K=1 bias row), writing Y = concat @ Wo + bo tile by tile.
  All matmuls run in float32r (1 col/cycle for N>=256 vs 4 for fp32).
"""

import numpy as np

import concourse.bass as bass
import concourse.mybir as mybir
import concourse.tile as tile
import concourse.bacc as bacc
from concourse import bass_utils

B, S, E, H, D = 8, 1024, 768, 12, 64
HD = H * D
P = 128
SCALE = 1.0 / np.float32(np.sqrt(D))
NKT = E // P     # 6 contraction chunks for the input projections
NMT = HD // P    # 6 head-major tiles (2 heads each)
NST = S // P     # 8 sequence tiles of 128
F32 = mybir.dt.float32
F32R = mybir.dt.float32r
MMDT = mybir.dt.float16
NPDT = np.float16
EXP = mybir.ActivationFunctionType.Exp

_CACHE = {}


def _build():
    nc = bacc.Bacc("TRN2", debug=False, target_bir_lowering=False)

    xqT = nc.dram_tensor("xqT", [E, S], MMDT, kind="ExternalInput").ap()
    xkT = nc.dram_tensor("xkT", [E, S], MMDT, kind="ExternalInput").ap()
    xvT = nc.dram_tensor("xvT", [E, S], MMDT, kind="ExternalInput").ap()
    wq = nc.dram_tensor("wq", [E, HD], MMDT, kind="ExternalInput").ap()
    wk = nc.dram_tensor("wk", [E, HD], MMDT, kind="ExternalInput").ap()
    wv = nc.dram_tensor("wv", [E + 1, HD], MMDT, kind="ExternalInput").ap()
    wo = nc.dram_tensor("wo", [HD + 1, E], MMDT, kind="ExternalInput").ap()
    bqk = nc.dram_tensor("bqk", [P, 2 * NMT], F32, kind="ExternalInput").ap()
    tri = nc.dram_tensor("tri", [P, P], MMDT, kind="ExternalInput").ap()
    ones_s = nc.dram_tensor("ones_s", [1, S], MMDT, kind="ExternalInput").ap()
    ones_d = nc.dram_tensor("ones_d", [1, 64], MMDT, kind="ExternalInput").ap()
    ones_c = nc.dram_tensor("ones_c", [P, H], MMDT, kind="ExternalInput").ap()
    y = nc.dram_tensor("y", [S, E], F32, kind="ExternalOutput").ap()

    with tile.TileContext(nc) as tc:
        with (
            tc.tile_pool(name="cst", bufs=1) as cst,
            tc.tile_pool(name="xin", bufs=12) as xin,
            tc.tile_pool(name="win", bufs=12) as win,
            tc.tile_pool(name="wbias", bufs=3) as wbias,
            tc.tile_pool(name="qt", bufs=2) as qtp,
            tc.tile_pool(name="kt", bufs=2) as ktp,
            tc.tile_pool(name="vp", bufs=8) as vpp,
            tc.tile_pool(name="numt", bufs=6) as numtp,
            tc.tile_pool(name="expt", bufs=4) as exptp,
            tc.tile_pool(name="dst", bufs=2) as dstp,
            tc.tile_pool(name="dinv", bufs=2) as dinvp,
            tc.tile_pool(name="outsb", bufs=2) as outp,
            tc.tile_pool(name="projps", bufs=2, space="PSUM") as projps,
            tc.tile_pool(name="scps", bufs=2, space="PSUM") as scps,
            tc.tile_pool(name="pvps", bufs=2, space="PSUM") as pvps,
        ):
            # ---- constants ----
            onr = cst.tile([1, S], MMDT, tag="onr")
            nc.sync.dma_start(onr[:], ones_s[:])
            on64 = cst.tile([P, 64], MMDT, tag="on64")
            nc.sync.dma_start(on64[64:65, :], ones_d[:])
            trit = cst.tile([P, P], MMDT, tag="trit")
            nc.sync.dma_start(trit[:], tri[:])
            bqk_sb = cst.tile([P, 2 * NMT], F32, tag="bqk")
            nc.sync.dma_start(bqk_sb[:], bqk[:])

            # ---- V weights + inputs ----
            wv_sb = []
            for kk in range(NKT):
                t = win.tile([P, HD], MMDT, tag="w")
                nc.sync.dma_start(t[:], wv[kk * P:(kk + 1) * P, :])
                wv_sb.append(t)
            wv_b = wbias.tile([1, HD], MMDT, tag="wb")
            nc.sync.dma_start(wv_b[:], wv[E:E + 1, :])
            xv_sb = []
            for kk in range(NKT):
                t = xin.tile([P, S], MMDT, tag="x")
                nc.sync.dma_start(t[:], xvT[kk * P:(kk + 1) * P, :])
                xv_sb.append(t)

            # ---- V projection -> vp tiles [S-tile, H*(D+1)] with ones cols ----
            vp_sb = []
            for mS in range(NST):
                vt = vpp.tile([P, H * (D + 1)], MMDT, tag="v")
                vp_sb.append(vt)
                v3 = vt[:].rearrange("p (h e) -> p h e", e=D + 1)
                nc.sync.dma_start(v3[:, :, D:D + 1], ones_c[:])
                for n in range(2):
                    w = 512 if n == 0 else HD - 512
                    h0 = n * (512 // D)
                    ps = projps.tile([P, 512], F32, tag="pp")
                    for kk in range(NKT):
                        nc.tensor.matmul(
                            ps[:, :w],
                            xv_sb[kk][:, mS * P:(mS + 1) * P],
                            wv_sb[kk][:, n * 512:n * 512 + w],
                            start=(kk == 0), stop=False)
                    nc.tensor.matmul(
                        ps[:, :w],
                        onr[0:1, mS * P:(mS + 1) * P],
                        wv_b[0:1, n * 512:n * 512 + w],
                        start=False, stop=True)
                    nc.vector.tensor_copy(
                        v3[:, h0:h0 + w // D, 0:D], ps[:, :w])

            # ---- Q/K weights + inputs (loaded up front; slots recycle) ----
            wq_sb, wk_sb = [], []
            for kk in range(NKT):
                t = win.tile([P, HD], MMDT, tag="w")
                nc.sync.dma_start(t[:], wq[kk * P:(kk + 1) * P, :])
                wq_sb.append(t)
            xq_sb = []
            for kk in range(NKT):
                t = xin.tile([P, S], MMDT, tag="x")
                nc.sync.dma_start(t[:], xqT[kk * P:(kk + 1) * P, :])
                xq_sb.append(t)
            for kk in range(NKT):
                t = win.tile([P, HD], MMDT, tag="w")
                nc.sync.dma_start(t[:], wk[kk * P:(kk + 1) * P, :])
                wk_sb.append(t)
            xk_sb = []
            for kk in range(NKT):
                t = xin.tile([P, S], MMDT, tag="x")
                nc.sync.dma_start(t[:], xkT[kk * P:(kk + 1) * P, :])
                xk_sb.append(t)

            def qk_proj(m, w_sb, bcol, x_sb, out_t, n):
                """One [128, 512] psum tile of the QT/KT projection."""
                ps = projps.tile([P, 512], F32, tag="pp")
                for kk in range(NKT):
                    nc.tensor.matmul(
                        ps[:],
                        w_sb[kk][:, m * P:(m + 1) * P],
                        x_sb[kk][:, n * 512:(n + 1) * 512],
                        start=(kk == 0), stop=(kk == NKT - 1))
                nc.vector.tensor_scalar(
                    out=out_t[:, n * 512:(n + 1) * 512], in0=ps[:],
                    scalar1=bqk_sb[:, bcol:bcol + 1], scalar2=None,
                    op0=mybir.AluOpType.add)

            numt_sb = [None] * NMT

            def attention_head(h, qt_t, kt_t, num_t, filler):
                """Causal attention for one head; filler() emits matmul work
                between stages to keep the PE busy while ACT runs exp."""
                off = (h % 2) * 64
                dinv = dinvp.tile([P, S], F32, tag="di")
                dtmp = dstp.tile([P, S], F32, tag="dt")
                for ci in range(2):
                    nblk = 4 * ci + 4
                    ets = []
                    # scores (transposed [t, s]) + exp, two t-blocks per tile
                    for tj in range(nblk // 2):
                        sp = scps.tile([P, 1024], F32, tag="sc")
                        et = exptp.tile([P, 1024], MMDT, tag="et")
                        ets.append(et)
                        for b2 in range(2):
                            ti = tj * 2 + b2
                            voff = max(0, ti * P - ci * 512)
                            col0 = min(voff, 256)  # widen to >=256 for f32r
                            nc.tensor.matmul(
                                sp[:, b2 * 512 + col0:(b2 + 1) * 512],
                                kt_t[off:off + 64, ti * P:(ti + 1) * P],
                                qt_t[off:off + 64,
                                     ci * 512 + col0:(ci + 1) * 512],
                                start=True, stop=True)
                        nc.scalar.activation(et[:], sp[:], EXP, scale=float(SCALE))
                        for b2 in range(2):
                            ti = tj * 2 + b2
                            voff = ti * P - ci * 512
                            if 0 <= voff < 512:  # diagonal block: triangular mask
                                sl = et[:, b2 * 512 + voff:b2 * 512 + voff + P]
                                nc.gpsimd.tensor_mul(sl, sl, trit[:])
                    if filler:
                        filler()
                    # P @ [V | 1]  (accumulate over t-blocks; den in row 64)
                    pvp = pvps.tile([65, 512], F32, tag="pv")
                    for ti in range(nblk):
                        voff = max(0, ti * P - ci * 512)
                        et = ets[ti // 2]
                        nc.tensor.matmul(
                            pvp[:, voff:512],
                            vp_sb[ti][:, h * (D + 1):(h + 1) * (D + 1)],
                            et[:, (ti % 2) * 512 + voff:(ti % 2 + 1) * 512],
                            start=(ti == 0), stop=(ti == nblk - 1),
                            skip_group_check=(ti > 0))
                    # numerator -> numt rows [off:off+64]
                    nc.vector.tensor_copy(
                        num_t[off:off + 64, ci * 512:(ci + 1) * 512],
                        pvp[0:64, :])
                    # denominator: stage row, broadcast via K=1 outer product
                    dstg = dstp.tile([P, 512], MMDT, tag="ds")
                    nc.vector.tensor_copy(dstg[64:65, :], pvp[64:65, :])
                    dps = projps.tile([P, 512], F32, tag="pp")
                    nc.tensor.matmul(
                        dps[0:64, :],
                        on64[64:65, 0:64],
                        dstg[64:65, :],
                        start=True, stop=True)
                    nc.vector.tensor_copy(
                        dtmp[0:64, ci * 512:(ci + 1) * 512], dps[0:64, :])
                nc.vector.reciprocal_approx_fast(
                    out=dinv[0:64, :], in_=dtmp[0:64, :])
                if off:
                    nc.vector.tensor_copy(dinv[64:128, :], dinv[0:64, :])
                # normalize this head's numerator rows
                nc.vector.tensor_mul(
                    num_t[off:off + 64, :], num_t[off:off + 64, :],
                    dinv[off:off + 64, :])

            # ---- interleaved Q/K projection + attention, pair by pair ----
            # pair 0 needs qt/kt tile 0 first
            qt_t = qtp.tile([P, S], MMDT, tag="q")
            kt_t = ktp.tile([P, S], MMDT, tag="k")
            for n in range(2):
                qk_proj(0, wq_sb, 0, xq_sb, qt_t, n)
            for n in range(2):
                qk_proj(0, wk_sb, NMT, xk_sb, kt_t, n)
            for m in range(NMT):
                num_t = numtp.tile([P, S], MMDT, tag="n")
                numt_sb[m] = num_t
                # filler work: next pair's projections, one psum tile at a time
                fill_jobs = []
                if m + 1 < NMT:
                    qt_n = qtp.tile([P, S], MMDT, tag="q")
                    kt_n = ktp.tile([P, S], MMDT, tag="k")
                    for n in range(2):
                        fill_jobs.append((qk_proj, m + 1, wq_sb, m + 1, xq_sb, qt_n, n))
                    for n in range(2):
                        fill_jobs.append((qk_proj, m + 1, wk_sb, NMT + m + 1, xk_sb, kt_n, n))

                def filler():
                    if fill_jobs:
                        f, *args = fill_jobs.pop(0)
                        f(*args)

                attention_head(2 * m, qt_t, kt_t, num_t, filler)
                attention_head(2 * m + 1, qt_t, kt_t, num_t, filler)
                while fill_jobs:
                    filler()
                if m + 1 < NMT:
                    qt_t, kt_t = qt_n, kt_n

            # ---- output projection ----
            wo_sb = []
            for kk in range(NMT):
                t = win.tile([P, E], MMDT, tag="w")
                nc.sync.dma_start(t[:], wo[kk * P:(kk + 1) * P, :])
                wo_sb.append(t)
            wo_b = wbias.tile([1, E], MMDT, tag="wb")
            nc.sync.dma_start(wo_b[:], wo[HD:HD + 1, :])
            for mS in range(NST):
                ot = outp.tile([P, E], F32, tag="o")
                for n in range(2):
                    w = 512 if n == 0 else E - 512
                    ps = projps.tile([P, 512], F32, tag="pp")
                    for kk in range(NMT):
                        nc.tensor.matmul(
                            ps[:, :w],
                            numt_sb[kk][:, mS * P:(mS + 1) * P],
                            wo_sb[kk][:, n * 512:n * 512 + w],
                            start=(kk == 0), stop=False)
                    nc.tensor.matmul(
                        ps[:, :w],
                        onr[0:1, mS * P:(mS + 1) * P],
                        wo_b[0:1, n * 512:n * 512 + w],
                        start=False, stop=True)
                    nc.vector.tensor_copy(ot[:, n * 512:n * 512 + w], ps[:, :w])
                nc.sync.dma_start(y[mS * P:(mS + 1) * P, :], ot[:])

    nc.compile()
    return nc


def _get_nc():
    if "nc" not in _CACHE:
        _CACHE["nc"] = _build()
    return _CACHE["nc"]


def kernel(queries, keys, values, attn_mask, Wq, bq, Wk, bk, Wv, bv, Wo, bo,
           **extra):
    nc = _get_nc()

    queries = np.asarray(queries, np.float32)
    keys = np.asarray(keys, np.float32)
    values = np.asarray(values, np.float32)
    wq_a = np.asarray(Wq, np.float32).transpose(1, 0, 2).reshape(E, HD).astype(NPDT)
    wk_a = np.asarray(Wk, np.float32).transpose(1, 0, 2).reshape(E, HD).astype(NPDT)
    wv_a = np.concatenate(
        [np.asarray(Wv, np.float32).transpose(1, 0, 2).reshape(E, HD),
         np.asarray(bv, np.float32).reshape(1, HD)], axis=0).astype(NPDT)
    wo_a = np.concatenate(
        [np.asarray(Wo, np.float32).reshape(HD, E),
         np.asarray(bo, np.float32).reshape(1, E)], axis=0).astype(NPDT)
    bq_f = np.asarray(bq, np.float32).reshape(HD)
    bk_f = np.asarray(bk, np.float32).reshape(HD)
    bqk_a = np.stack([bq_f[m * P:(m + 1) * P] for m in range(NMT)]
                     + [bk_f[m * P:(m + 1) * P] for m in range(NMT)],
                     axis=1).astype(np.float32)
    tri_m = np.triu(np.ones((P, P), NPDT))
    ones_s = np.ones((1, S), NPDT)
    ones_d = np.ones((1, 64), NPDT)
    ones_c = np.ones((P, H), NPDT)

    in_maps = []
    for b in range(B):
        in_maps.append({
            "xqT": np.ascontiguousarray(queries[b].T).astype(NPDT),
            "xkT": np.ascontiguousarray(keys[b].T).astype(NPDT),
            "xvT": np.ascontiguousarray(values[b].T).astype(NPDT),
            "wq": wq_a, "wk": wk_a, "wv": wv_a, "wo": wo_a, "bqk": bqk_a,
            "tri": tri_m, "ones_s": ones_s, "ones_d": ones_d,
            "ones_c": ones_c,
        })
    res = bass_utils.run_bass_kernel_spmd(
        nc, in_maps, core_ids=list(range(B)), **extra)
    out = np.stack([res.results[c]["y"] for c in range(B)], axis=0)
    if extra:
        _CACHE["last_result"] = res
    return out


# revision 6
# speedup vs baseline: 1.4186x; 1.0287x over previous
"""Trainium2 Bass kernel: decoder multi-head attention (causal), batch-parallel
across 8 NeuronCores (one batch element per core).

Per-core layout (S=1024, E=768, H=12, D=64):
  - host pre-transposes inputs to [E, S] fp16; per-head projection weights
    packed to [E, H*D] fp16.
  - QT/KT computed in [H*D, S] layout (2 heads per 128-partition tile);
    Q/K biases added at PSUM eviction as per-partition scalars.
  - V computed in natural [S, H*(D+1)] layout with a ones column per head so
    P@[V|1] also produces the softmax denominator (psum row 64); V bias is
    added at eviction from a broadcast-row tile.
  - scores computed transposed [t, s] per 128-row t-block; softmax skips the
    max subtraction (scores are bounded); exp on ScalarE with fused *SCALE;
    causal diagonal-block masking via GpSimd multiply with a triangular mask;
    fully-masked column ranges are never read by the P@[V|1] matmuls.
  - the two heads of a partition tile are processed interleaved so their
    K=64 score matmuls sit on different PE row groups and exp/PV overlap.
  - denominator broadcast via a K=1 outer-product matmul, reciprocal via
    DVE reciprocal_approx_fast, per-head normalize of the [H*D, S] numerator.
  - output projection consumes the normalized numerator directly; out bias
    is added at eviction from a broadcast-row tile.
  All matmuls run on fp16 inputs with fp32 PSUM accumulation.
"""

import numpy as np

import concourse.bass as bass
import concourse.mybir as mybir
import concourse.tile as tile
import concourse.bacc as bacc
from concourse import bass_utils

B, S, E, H, D = 8, 1024, 768, 12, 64
HD = H * D
P = 128
SCALE = 1.0 / np.float32(np.sqrt(D))
NKT = E // P     # 6 contraction chunks for the input projections
NMT = HD // P    # 6 head-major tiles (2 heads each)
NST = S // P     # 8 sequence tiles of 128
F32 = mybir.dt.float32
MMDT = mybir.dt.float16
NPDT = np.float16
EXP = mybir.ActivationFunctionType.Exp
ADD = mybir.AluOpType.add

_CACHE = {}


def _build():
    nc = bacc.Bacc("TRN2", debug=False, target_bir_lowering=False)

    xqT = nc.dram_tensor("xqT", [E, S], MMDT, kind="ExternalInput").ap()
    xkT = nc.dram_tensor("xkT", [E, S], MMDT, kind="ExternalInput").ap()
    xvT = nc.dram_tensor("xvT", [E, S], MMDT, kind="ExternalInput").ap()
    wq = nc.dram_tensor("wq", [E, HD], MMDT, kind="ExternalInput").ap()
    wk = nc.dram_tensor("wk", [E, HD], MMDT, kind="ExternalInput").ap()
    wv = nc.dram_tensor("wv", [E, HD], MMDT, kind="ExternalInput").ap()
    wo = nc.dram_tensor("wo", [HD, E], MMDT, kind="ExternalInput").ap()
    bqk = nc.dram_tensor("bqk", [P, 2 * NMT], F32, kind="ExternalInput").ap()
    bvb = nc.dram_tensor("bvb", [P, HD], F32, kind="ExternalInput").ap()
    bob = nc.dram_tensor("bob", [P, E], F32, kind="ExternalInput").ap()
    tri = nc.dram_tensor("tri", [P, P], MMDT, kind="ExternalInput").ap()
    ones_d = nc.dram_tensor("ones_d", [1, 64], MMDT, kind="ExternalInput").ap()
    ones_c = nc.dram_tensor("ones_c", [P, H], MMDT, kind="ExternalInput").ap()
    y = nc.dram_tensor("y", [S, E], F32, kind="ExternalOutput").ap()

    with tile.TileContext(nc) as tc:
        with (
            tc.tile_pool(name="cst", bufs=1) as cst,
            tc.tile_pool(name="xin", bufs=12) as xin,
            tc.tile_pool(name="win", bufs=12) as win,
            tc.tile_pool(name="qt", bufs=3) as qtp,
            tc.tile_pool(name="kt", bufs=3) as ktp,
            tc.tile_pool(name="vp", bufs=8) as vpp,
            tc.tile_pool(name="numt", bufs=6) as numtp,
            tc.tile_pool(name="expt", bufs=10) as exptp,
            tc.tile_pool(name="dst", bufs=3) as dstp,
            tc.tile_pool(name="dinv", bufs=3) as dinvp,
            tc.tile_pool(name="outsb", bufs=3) as outp,
            tc.tile_pool(name="projps", bufs=2, space="PSUM") as projps,
            tc.tile_pool(name="scps", bufs=2, space="PSUM") as scps,
            tc.tile_pool(name="pvps", bufs=2, space="PSUM") as pvps,
        ):
            # ---- constants ----
            on64 = cst.tile([P, 64], MMDT, tag="on64")
            nc.sync.dma_start(on64[64:65, :], ones_d[:])
            trit = cst.tile([P, P], MMDT, tag="trit")
            nc.sync.dma_start(trit[:], tri[:])
            bqk_sb = cst.tile([P, 2 * NMT], F32, tag="bqk")
            nc.sync.dma_start(bqk_sb[:], bqk[:])
            bvb_sb = cst.tile([P, HD], F32, tag="bvb")
            nc.sync.dma_start(bvb_sb[:], bvb[:])
            bob_sb = cst.tile([P, E], F32, tag="bob")
            nc.sync.dma_start(bob_sb[:], bob[:])

            # ---- V weights + inputs ----
            wv_sb = []
            for kk in range(NKT):
                t = win.tile([P, HD], MMDT, tag="w")
                nc.sync.dma_start(t[:], wv[kk * P:(kk + 1) * P, :])
                wv_sb.append(t)
            xv_sb = []
            for kk in range(NKT):
                t = xin.tile([P, S], MMDT, tag="x")
                nc.sync.dma_start(t[:], xvT[kk * P:(kk + 1) * P, :])
                xv_sb.append(t)

            # ---- V projection -> vp tiles [S-tile, H*(D+1)] with ones cols ----
            vp_sb = []
            for mS in range(NST):
                vt = vpp.tile([P, H * (D + 1)], MMDT, tag="v")
                vp_sb.append(vt)
                v3 = vt[:].rearrange("p (h e) -> p h e", e=D + 1)
                nc.sync.dma_start(v3[:, :, D:D + 1], ones_c[:])
                for n in range(2):
                    w = 512 if n == 0 else HD - 512
                    h0 = n * (512 // D)
                    ps = projps.tile([P, 512], F32, tag="pp")
                    for kk in range(NKT):
                        nc.tensor.matmul(
                            ps[:, :w],
                            xv_sb[kk][:, mS * P:(mS + 1) * P],
                            wv_sb[kk][:, n * 512:n * 512 + w],
                            start=(kk == 0), stop=(kk == NKT - 1))
                    nc.vector.tensor_tensor(
                        v3[:, h0:h0 + w // D, 0:D], ps[:, :w],
                        bvb_sb[:, n * 512:n * 512 + w], ADD)

            # ---- Q/K weights + inputs (loaded up front; slots recycle) ----
            wq_sb, wk_sb = [], []
            for kk in range(NKT):
                t = win.tile([P, HD], MMDT, tag="w")
                nc.sync.dma_start(t[:], wq[kk * P:(kk + 1) * P, :])
                wq_sb.append(t)
            xq_sb = []
            for kk in range(NKT):
                t = xin.tile([P, S], MMDT, tag="x")
                nc.sync.dma_start(t[:], xqT[kk * P:(kk + 1) * P, :])
                xq_sb.append(t)
            for kk in range(NKT):
                t = win.tile([P, HD], MMDT, tag="w")
                nc.sync.dma_start(t[:], wk[kk * P:(kk + 1) * P, :])
                wk_sb.append(t)
            xk_sb = []
            for kk in range(NKT):
                t = xin.tile([P, S], MMDT, tag="x")
                nc.sync.dma_start(t[:], xkT[kk * P:(kk + 1) * P, :])
                xk_sb.append(t)

            def qk_proj(m, w_sb, bcol, x_sb, out_t, n):
                """One [128, 512] psum tile of the QT/KT projection."""
                ps = projps.tile([P, 512], F32, tag="pp")
                for kk in range(NKT):
                    nc.tensor.matmul(
                        ps[:],
                        w_sb[kk][:, m * P:(m + 1) * P],
                        x_sb[kk][:, n * 512:(n + 1) * 512],
                        start=(kk == 0), stop=(kk == NKT - 1))
                nc.vector.tensor_scalar(
                    out=out_t[:, n * 512:(n + 1) * 512], in0=ps[:],
                    scalar1=bqk_sb[:, bcol:bcol + 1], scalar2=None,
                    op0=ADD)

            numt_sb = [None] * NMT

            def attention_pair(m, qt_t, kt_t, num_t, filler):
                """Both heads of tile m, interleaved for PE/ACT overlap."""
                dts, dis = [], []
                for h in (0, 1):
                    dt_ = dstp.tile([P, S], F32, tag="dt", name=f"dtmp{m}_{h}")
                    dts.append(dt_)
                    di_ = dinvp.tile([P, S], F32, tag="di", name=f"dinv{m}_{h}")
                    dis.append(di_)
                for ci in range(2):
                    nblk = 4 * ci + 4
                    ets = {0: [], 1: []}
                    # scores + exp, alternating heads per 2-block tile
                    for tj in range(nblk // 2):
                        for h in (0, 1):
                            off = h * 64
                            sp = scps.tile([P, 1024], F32, tag="sc",
                                           name=f"sp{m}_{h}_{ci}_{tj}")
                            et = exptp.tile([P, 1024], MMDT, tag="et",
                                            name=f"et{m}_{h}_{ci}_{tj}")
                            ets[h].append(et)
                            for b2 in range(2):
                                ti = tj * 2 + b2
                                voff = max(0, ti * P - ci * 512)
                                col0 = min(voff, 256)
                                nc.tensor.matmul(
                                    sp[:, b2 * 512 + col0:(b2 + 1) * 512],
                                    kt_t[off:off + 64, ti * P:(ti + 1) * P],
                                    qt_t[off:off + 64,
                                         ci * 512 + col0:(ci + 1) * 512],
                                    start=True, stop=True)
                            nc.scalar.activation(et[:], sp[:], EXP,
                                                 scale=float(SCALE))
                            for b2 in range(2):
                                ti = tj * 2 + b2
                                voff = ti * P - ci * 512
                                if 0 <= voff < 512:  # diagonal block
                                    sl = et[:, b2 * 512 + voff:
                                            b2 * 512 + voff + P]
                                    nc.gpsimd.tensor_mul(sl, sl, trit[:])
                        if filler and tj % 2 == 1:
                            filler()
                    # P @ [V|1] per head (den lands in psum row 64)
                    for h in (0, 1):
                        off = h * 64
                        hh = 2 * m + h
                        pvp = pvps.tile([65, 512], F32, tag="pv",
                                        name=f"pv{m}_{h}_{ci}")
                        for ti in range(nblk):
                            voff = max(0, ti * P - ci * 512)
                            et = ets[h][ti // 2]
                            nc.tensor.matmul(
                                pvp[:, voff:512],
                                vp_sb[ti][:, hh * (D + 1):(hh + 1) * (D + 1)],
                                et[:, (ti % 2) * 512 + voff:(ti % 2 + 1) * 512],
                                start=(ti == 0), stop=(ti == nblk - 1),
                                skip_group_check=(ti > 0))
                        nc.vector.tensor_copy(
                            num_t[off:off + 64, ci * 512:(ci + 1) * 512],
                            pvp[0:64, :])
                        dstg = dstp.tile([P, 512], MMDT, tag="ds",
                                         name=f"ds{m}_{h}_{ci}")
                        nc.vector.tensor_copy(dstg[64:65, :], pvp[64:65, :])
                        dps = projps.tile([P, 512], F32, tag="pp",
                                          name=f"dps{m}_{h}_{ci}")
                        nc.tensor.matmul(dps[0:64, :], on64[64:65, 0:64],
                                         dstg[64:65, :], start=True, stop=True)
                        nc.vector.tensor_copy(
                            dts[h][0:64, ci * 512:(ci + 1) * 512], dps[0:64, :])
                    if filler:
                        filler()
                # reciprocal + normalize
                for h in (0, 1):
                    off = h * 64
                    nc.vector.reciprocal_approx_fast(
                        out=dis[h][0:64, :], in_=dts[h][0:64, :])
                    if off:
                        nc.vector.tensor_copy(dis[h][64:128, :], dis[h][0:64, :])
                    nc.vector.tensor_mul(
                        num_t[off:off + 64, :], num_t[off:off + 64, :],
                        dis[h][off:off + 64, :])

            # ---- interleaved Q/K projection + attention, pair by pair ----
            qt_t = qtp.tile([P, S], MMDT, tag="q")
            kt_t = ktp.tile([P, S], MMDT, tag="k")
            for n in range(2):
                qk_proj(0, wq_sb, 0, xq_sb, qt_t, n)
            for n in range(2):
                qk_proj(0, wk_sb, NMT, xk_sb, kt_t, n)
            for m in range(NMT):
                num_t = numtp.tile([P, S], MMDT, tag="n")
                numt_sb[m] = num_t
                fill_jobs = []
                if m + 1 < NMT:
                    qt_n = qtp.tile([P, S], MMDT, tag="q")
                    kt_n = ktp.tile([P, S], MMDT, tag="k")
                    for n in range(2):
                        fill_jobs.append((qk_proj, m + 1, wq_sb, m + 1,
                                          xq_sb, qt_n, n))
                    for n in range(2):
                        fill_jobs.append((qk_proj, m + 1, wk_sb, NMT + m + 1,
                                          xk_sb, kt_n, n))

                def filler():
                    if fill_jobs:
                        f, *args = fill_jobs.pop(0)
                        f(*args)

                attention_pair(m, qt_t, kt_t, num_t, filler)
                while fill_jobs:
                    filler()
                if m + 1 < NMT:
                    qt_t, kt_t = qt_n, kt_n

            # ---- output projection ----
            wo_sb = []
            for kk in range(NMT):
                t = win.tile([P, E], MMDT, tag="w")
                nc.sync.dma_start(t[:], wo[kk * P:(kk + 1) * P, :])
                wo_sb.append(t)
            for mS in range(NST):
                ot = outp.tile([P, E], F32, tag="o")
                for n in range(2):
                    w = 512 if n == 0 else E - 512
                    ps = projps.tile([P, 512], F32, tag="pp")
                    for kk in range(NMT):
                        nc.tensor.matmul(
                            ps[:, :w],
                            numt_sb[kk][:, mS * P:(mS + 1) * P],
                            wo_sb[kk][:, n * 512:n * 512 + w],
                            start=(kk == 0), stop=(kk == NMT - 1))
                    nc.vector.tensor_tensor(
                        ot[:, n * 512:n * 512 + w], ps[:, :w],
                        bob_sb[:, n * 512:n * 512 + w], ADD)
                nc.sync.dma_start(y[mS * P:(mS + 1) * P, :], ot[:])

    nc.compile()
    return nc


def _get_nc():
    if "nc" not in _CACHE:
        _CACHE["nc"] = _build()
    return _CACHE["nc"]


def kernel(queries, keys, values, attn_mask, Wq, bq, Wk, bk, Wv, bv, Wo, bo,
           **extra):
    nc = _get_nc()

    queries = np.asarray(queries, np.float32)
    keys = np.asarray(keys, np.float32)
    values = np.asarray(values, np.float32)
    wq_a = np.asarray(Wq, np.float32).transpose(1, 0, 2).reshape(E, HD).astype(NPDT)
    wk_a = np.asarray(Wk, np.float32).transpose(1, 0, 2).reshape(E, HD).astype(NPDT)
    wv_a = np.asarray(Wv, np.float32).transpose(1, 0, 2).reshape(E, HD).astype(NPDT)
    wo_a = np.asarray(Wo, np.float32).reshape(HD, E).astype(NPDT)
    bq_f = np.asarray(bq, np.float32).reshape(HD)
    bk_f = np.asarray(bk, np.float32).reshape(HD)
    bqk_a = np.stack([bq_f[m * P:(m + 1) * P] for m in range(NMT)]
                     + [bk_f[m * P:(m + 1) * P] for m in range(NMT)],
                     axis=1).astype(np.float32)
    bvb_a = np.ascontiguousarray(np.broadcast_to(
        np.asarray(bv, np.float32).reshape(1, HD), (P, HD)))
    bob_a = np.ascontiguousarray(np.broadcast_to(
        np.asarray(bo, np.float32).reshape(1, E), (P, E)))
    tri_m = np.triu(np.ones((P, P), NPDT))
    ones_d = np.ones((1, 64), NPDT)
    ones_c = np.ones((P, H), NPDT)

    in_maps = []
    for b in range(B):
        in_maps.append({
            "xqT": np.ascontiguousarray(queries[b].T).astype(NPDT),
            "xkT": np.ascontiguousarray(keys[b].T).astype(NPDT),
            "xvT": np.ascontiguousarray(values[b].T).astype(NPDT),
            "wq": wq_a, "wk": wk_a, "wv": wv_a, "wo": wo_a,
            "bqk": bqk_a, "bvb": bvb_a, "bob": bob_a,
            "tri": tri_m, "ones_d": ones_d, "ones_c": ones_c,
        })
    res = bass_utils.run_bass_kernel_spmd(
        nc, in_maps, core_ids=list(range(B)), **extra)
    out = np.stack([res.results[c]["y"] for c in range(B)], axis=0)
    if extra:
        _CACHE["last_result"] = res
    return out
